# revision 9
# baseline (speedup 1.0000x reference)
"""Trainium2 Bass kernel for AnalyzableUniformAttention.

Reference computation (per batch element):
    v     = (x @ Wv.T) @ Wo.T
    mixed = T @ v            with T[i,j] = 1/(i+1) for j<=i  (causal cumulative mean)
    out   = x + mixed

Key algebraic restructure: T acts on the sequence dim, the projections act on
the feature dim, so they commute:
    out = x + D^-1 @ (tril @ x) @ W2      with W2 = Wv.T @ Wo.T, D = diag(i+1)

This halves the matmul FLOPs (one HxH projection instead of two) and lets the
prefix-sum (tril @ x) be computed block-wise with staircase matmuls plus an
f32 per-block running-total carry.

Sharding: pure data-parallel over batch B=16 -> 2 batch elements per core on
8 NeuronCores; the HxH weights are replicated. No collectives.

Per-core schedule (L=4096 tokens, H=256), all matmuls in float32r (full-rate
when the moving dim is 256):
  - tokens processed in blocks of 256; partition p holds the two ADJACENT
    tokens {256g+2p, 256g+2p+1} of block g, so every DMA descriptor moves a
    2 KiB contiguous HBM run. DMA at 1 MiB granularity (4 blocks), in-DMAs
    issued on the sync HWDGE ring, out-DMAs on the (otherwise idle) GpSimd
    SWDGE ring so they never head-of-line-block the input stream.
  - block prefix:  psa[:, hf*256+i] = sum_p x[2p+tt, h] * R_tt[p, i] for the
    even/odd token groups tt (R_tt[p,i] = [2p+tt <= i]), accumulated in one
    PSUM bank; psa column i is block token i in order.
  - carry (ACT):   prefT = psa + carry[h]; the carry scalar is simply the
                   previous block's prefT column 255 (running total).
  - projection:    psP_t[i, h'] = sum_h prefT[h, 2i+t] * W2[h, h'] via
                   stride-2 lhsT column slices, so psP_t partition p is block
                   token 2p+t — matching the DMA-friendly output layout.
  - scale+residual fused on DVE: out = (psP * recip_i) + x.
"""

import sys

import numpy as np

sys.path.insert(0, "/opt/trn_rl_repo")

import concourse.bacc as bacc  # noqa: E402
import concourse.bass as bass  # noqa: E402
import concourse.tile as tile  # noqa: E402
from concourse import mybir  # noqa: E402
from concourse.bass_utils import run_bass_kernel_spmd  # noqa: E402

B, L, H = 16, 4096, 256
NCORES = 8
BPC = B // NCORES  # batch elements per core
P = 128  # partitions
BT = 2 * P  # tokens per block (256)
NBLOCK = L // BT  # 16 blocks per batch element
KSC = 2  # blocks per superchunk (512 KiB DMA granularity)
NSUPER = NBLOCK // KSC

F32 = mybir.dt.float32
F32R = mybir.dt.float32r
AF = mybir.ActivationFunctionType
ALU = mybir.AluOpType


def build() -> bass.Bass:
    nc = bacc.Bacc()
    x = nc.declare_dram_parameter("x", [BPC, L, H], F32R, isOutput=False)
    wv = nc.declare_dram_parameter("wv", [H, H], F32R, isOutput=False)
    wot = nc.declare_dram_parameter("wot", [H, H], F32R, isOutput=False)
    # rmat = [R_even | R_odd]: R_tt[p, i] = 1 iff 2p+tt <= i
    rmat = nc.declare_dram_parameter("rmat", [P, 2 * BT], F32R, isOutput=False)
    # recips[p, 2g+t] = 1/(256g + 2p + t + 1)
    recips = nc.declare_dram_parameter("recips", [P, 2 * NBLOCK], F32, isOutput=False)
    out = nc.declare_dram_parameter("out", [BPC, L, H], F32, isOutput=True)

    with tile.TileContext(nc) as tc:
        with (
            tc.tile_pool(name="singles", bufs=1) as singles,
            tc.tile_pool(name="xpool", bufs=6) as xpool,
            tc.tile_pool(name="opool", bufs=4) as opool,
            tc.tile_pool(name="prefpool", bufs=6) as prefpool,
            tc.tile_pool(name="psumA", bufs=4, space="PSUM") as psuma_pool,
            tc.tile_pool(name="psumP", bufs=4, space="PSUM") as psump_pool,
        ):
            # token t = s*1024 + k*256 + 2p + two
            xr = x[:].rearrange("b (s k p two) h -> b s p k two h", k=KSC, p=P, two=2)
            outr = out[:].rearrange(
                "b (s k p two) h -> b s p k two h", k=KSC, p=P, two=2
            )

            # ---- prefetch the first two superchunks before anything else so
            # compute can start as soon as the constants land ----
            xx_pending = []
            for pre in range(3):
                xx = xpool.tile([P, KSC, 2, H], F32R, tag="xx")
                nc.sync.dma_start(out=xx, in_=xr[0, pre])
                xx_pending.append(xx)

            # ---- one-time: constants + fused weight W2 = Wv.T @ Wo.T ----
            rmat_sb = singles.tile([P, 2 * BT], F32R, tag="rmat")
            nc.scalar.dma_start(out=rmat_sb, in_=rmat[:, :])
            recips_sb = singles.tile([P, 2 * NBLOCK], F32, tag="recips")
            nc.scalar.dma_start(out=recips_sb, in_=recips[:, :])
            zero_sb = singles.tile([P, 1], F32, tag="zero")
            nc.vector.memset(zero_sb, 0.0)

            wvf, wotf = [], []
            for k in range(2):
                wv_f = singles.tile([P, H], F32R, tag=f"wvf{k}")
                nc.scalar.dma_start(out=wv_f, in_=wv[k * P : (k + 1) * P, :])
                wvf.append(wv_f)
                wot_f = singles.tile([P, H], F32R, tag=f"wotf{k}")
                nc.scalar.dma_start(out=wot_f, in_=wot[k * P : (k + 1) * P, :])
                wotf.append(wot_f)

            # W2[h,h'] = sum_g Wv[g,h] * WoT[g,h'] ; h-halves in partitions
            w2sb = []
            for mh in range(2):
                pw = psuma_pool.tile([P, H], F32, tag="psumA")
                nc.tensor.matmul(
                    pw, lhsT=wvf[0][:, mh * P : (mh + 1) * P], rhs=wotf[0][:, :],
                    start=True, stop=False,
                )
                nc.tensor.matmul(
                    pw, lhsT=wvf[1][:, mh * P : (mh + 1) * P], rhs=wotf[1][:, :],
                    start=False, stop=True,
                )
                w2 = singles.tile([P, H], F32R, tag=f"w2sb{mh}")
                nc.scalar.copy(w2, pw)
                w2sb.append(w2)

            # ---- main loop ----
            pre_iter = 3  # superchunks already prefetched (batch 0)
            for b in range(BPC):
                prev_pref = [None, None]  # prefT of previous block per h-half
                for s in range(NSUPER):
                    if b == 0 and s < pre_iter:
                        xx = xx_pending[s]
                    else:
                        xx = xpool.tile([P, KSC, 2, H], F32R, tag="xx")
                        nc.sync.dma_start(out=xx, in_=xr[b, s])
                    ot = opool.tile([P, KSC, 2, H], F32, tag="ot")
                    for k in range(KSC):
                        g = s * KSC + k  # global block idx within batch

                        # block prefix, transposed: psa[:, hf*BT+i] per h-half
                        psa = psuma_pool.tile([P, 2 * BT], F32, tag="psumA")
                        for hf in range(2):
                            nc.tensor.matmul(
                                psa[:, hf * BT : hf * BT + BT],
                                lhsT=xx[:, k, 0, hf * P : (hf + 1) * P],
                                rhs=rmat_sb[:, 0:BT],
                                start=True, stop=False,
                            )
                            nc.tensor.matmul(
                                psa[:, hf * BT : hf * BT + BT],
                                lhsT=xx[:, k, 1, hf * P : (hf + 1) * P],
                                rhs=rmat_sb[:, BT : 2 * BT],
                                start=False, stop=True,
                            )

                        # carry add on ACT (both halves; chains stay ACT-local);
                        # the running total IS the previous block's column 255
                        pref = []
                        for hf in range(2):
                            pr = prefpool.tile([P, BT], F32R, tag=f"pref{hf}")
                            if g == 0:
                                old = zero_sb[:, 0:1]
                            else:
                                old = prev_pref[hf][:, BT - 1 : BT].bitcast(F32)
                            nc.scalar.activation(
                                pr, psa[:, hf * BT : hf * BT + BT],
                                AF.Identity, bias=old,
                            )
                            pref.append(pr)
                        prev_pref = pref

                        # projection back to token-major (even/odd token groups
                        # in one PSUM bank) + fused scale+residual on DVE
                        pp = psump_pool.tile([P, 2 * H], F32, tag="psumP")
                        for t in range(2):
                            nc.tensor.matmul(
                                pp[:, t * H : (t + 1) * H],
                                lhsT=pref[0][:, t : BT : 2],
                                rhs=w2sb[0][:, :],
                                start=True, stop=False,
                            )
                            nc.tensor.matmul(
                                pp[:, t * H : (t + 1) * H],
                                lhsT=pref[1][:, t : BT : 2],
                                rhs=w2sb[1][:, :],
                                start=False, stop=True,
                            )
                        for t in range(2):
                            nc.vector.scalar_tensor_tensor(
                                ot[:, k, t, :],
                                pp[:, t * H : (t + 1) * H],
                                recips_sb[:, 2 * g + t : 2 * g + t + 1],
                                xx[:, k, t, :].bitcast(F32),
                                ALU.mult,
                                ALU.add,
                            )
                    # out-DMA on the idle GpSimd SWDGE ring (no HOL blocking
                    # of the input stream on the sync ring)
                    nc.gpsimd.dma_start(out=outr[b, s], in_=ot)
    nc.finalize()
    return nc


_NC_CACHE = None


def _get_nc() -> bass.Bass:
    global _NC_CACHE
    if _NC_CACHE is None:
        _NC_CACHE = build()
    return _NC_CACHE


def _make_in_maps(x, Wv, Wo):
    x = np.ascontiguousarray(np.asarray(x, dtype=np.float32))
    Wv = np.ascontiguousarray(np.asarray(Wv, dtype=np.float32))
    WoT = np.ascontiguousarray(np.asarray(Wo, dtype=np.float32).T)
    i_idx = np.arange(BT)[None, :]
    p_idx = np.arange(P)[:, None]
    r_even = (2 * p_idx <= i_idx).astype(np.float32)
    r_odd = (2 * p_idx + 1 <= i_idx).astype(np.float32)
    rmat_np = np.concatenate([r_even, r_odd], axis=1)  # [128, 512]
    # recips[p, 2g+t] = 1/(256g + 2p + t + 1)
    g_idx = np.arange(NBLOCK)[None, :]
    rec = np.zeros((P, 2 * NBLOCK), dtype=np.float32)
    rec[:, 0::2] = 1.0 / (256 * g_idx + 2 * p_idx + 1)
    rec[:, 1::2] = 1.0 / (256 * g_idx + 2 * p_idx + 2)
    return [
        {
            "x": x[BPC * i : BPC * (i + 1)],
            "wv": Wv,
            "wot": WoT,
            "rmat": rmat_np,
            "recips": rec,
        }
        for i in range(NCORES)
    ]


def run(x, Wv, Wo, trace: bool = False):
    """Run the SPMD kernel; returns (full_output, BassKernelResults)."""
    nc = _get_nc()
    in_maps = _make_in_maps(x, Wv, Wo)
    bres = run_bass_kernel_spmd(
        nc, in_maps, core_ids=list(range(NCORES)), trace=trace
    )
    full = np.concatenate(
        [np.asarray(bres.results[i]["out"]) for i in range(NCORES)], axis=0
    )
    return full, bres


def kernel(x, Wv, Wo) -> np.ndarray:
    return run(x, Wv, Wo)[0]


# revision 12
# speedup vs baseline: 1.0162x; 1.0162x over previous
"""Trainium2 Bass kernel for AnalyzableUniformAttention.

Reference computation (per batch element):
    v     = (x @ Wv.T) @ Wo.T
    mixed = T @ v            with T[i,j] = 1/(i+1) for j<=i  (causal cumulative mean)
    out   = x + mixed

Key algebraic restructure: T acts on the sequence dim, the projections act on
the feature dim, so they commute:
    out = x + D^-1 @ (tril @ x) @ W2      with W2 = Wv.T @ Wo.T, D = diag(i+1)

This halves the matmul FLOPs (one HxH projection instead of two) and lets the
prefix-sum (tril @ x) be computed block-wise with staircase matmuls plus an
f32 per-block running-total carry.

Sharding: pure data-parallel over batch B=16 -> 2 batch elements per core on
8 NeuronCores; the HxH weights are replicated. No collectives.

Per-core schedule (L=4096 tokens, H=256), all matmuls in float32r (full-rate
when the moving dim is 256):
  - tokens processed in blocks of 256; partition p holds the two ADJACENT
    tokens {256g+2p, 256g+2p+1} of block g, so every DMA descriptor moves a
    2 KiB contiguous HBM run. DMA at 1 MiB granularity (4 blocks), in-DMAs
    issued on the sync HWDGE ring, out-DMAs on the (otherwise idle) GpSimd
    SWDGE ring so they never head-of-line-block the input stream.
  - block prefix:  psa[:, hf*256+i] = sum_p x[2p+tt, h] * R_tt[p, i] for the
    even/odd token groups tt (R_tt[p,i] = [2p+tt <= i]), accumulated in one
    PSUM bank; psa column i is block token i in order.
  - carry (ACT):   prefT = psa + carry[h]; the carry scalar is simply the
                   previous block's prefT column 255 (running total).
  - projection:    psP_t[i, h'] = sum_h prefT[h, 2i+t] * W2[h, h'] via
                   stride-2 lhsT column slices, so psP_t partition p is block
                   token 2p+t — matching the DMA-friendly output layout.
  - scale+residual fused on DVE: out = (psP * recip_i) + x.
"""

import sys

import numpy as np

sys.path.insert(0, "/opt/trn_rl_repo")

import concourse.bacc as bacc  # noqa: E402
import concourse.bass as bass  # noqa: E402
import concourse.tile as tile  # noqa: E402
from concourse import mybir  # noqa: E402
from concourse.bass_utils import run_bass_kernel_spmd  # noqa: E402

B, L, H = 16, 4096, 256
NCORES = 8
BPC = B // NCORES  # batch elements per core
P = 128  # partitions
BT = 2 * P  # tokens per block (256)
NBLOCK = L // BT  # 16 blocks per batch element
KSC = 2  # blocks per superchunk (512 KiB DMA granularity)
NSUPER = NBLOCK // KSC

F32 = mybir.dt.float32
F32R = mybir.dt.float32r
AF = mybir.ActivationFunctionType
ALU = mybir.AluOpType


def build() -> bass.Bass:
    nc = bacc.Bacc()
    x = nc.declare_dram_parameter("x", [BPC, L, H], F32R, isOutput=False)
    wv = nc.declare_dram_parameter("wv", [H, H], F32R, isOutput=False)
    wot = nc.declare_dram_parameter("wot", [H, H], F32R, isOutput=False)
    # rmat = [R_even | R_odd]: R_tt[p, i] = 1 iff 2p+tt <= i
    rmat = nc.declare_dram_parameter("rmat", [P, 2 * BT], F32R, isOutput=False)
    # recips[p, 2g+t] = 1/(256g + 2p + t + 1)
    recips = nc.declare_dram_parameter("recips", [P, 2 * NBLOCK], F32, isOutput=False)
    out = nc.declare_dram_parameter("out", [BPC, L, H], F32, isOutput=True)

    with tile.TileContext(nc) as tc:
        with (
            tc.tile_pool(name="singles", bufs=1) as singles,
            tc.tile_pool(name="xpool", bufs=6) as xpool,
            tc.tile_pool(name="opool", bufs=4) as opool,
            tc.tile_pool(name="prefpool", bufs=6) as prefpool,
            tc.tile_pool(name="psumA", bufs=4, space="PSUM") as psuma_pool,
            tc.tile_pool(name="psumP", bufs=4, space="PSUM") as psump_pool,
        ):
            # token t = s*1024 + k*256 + 2p + two
            xr = x[:].rearrange("b (s k p two) h -> b s p k two h", k=KSC, p=P, two=2)
            outr = out[:].rearrange(
                "b (s k p two) h -> b s p k two h", k=KSC, p=P, two=2
            )

            # ---- prefetch the first two superchunks before anything else so
            # compute can start as soon as the constants land ----
            xx_pending = []
            for pre in range(3):
                xx = xpool.tile([P, KSC, 2, H], F32R, tag="xx")
                nc.sync.dma_start(out=xx, in_=xr[0, pre])
                xx_pending.append(xx)

            # ---- one-time: constants + fused weight W2 = Wv.T @ Wo.T ----
            rmat_sb = singles.tile([P, 2 * BT], F32R, tag="rmat")
            nc.scalar.dma_start(out=rmat_sb, in_=rmat[:, :])
            recips_sb = singles.tile([P, 2 * NBLOCK], F32, tag="recips")
            nc.scalar.dma_start(out=recips_sb, in_=recips[:, :])
            zero_sb = singles.tile([P, 1], F32, tag="zero")
            nc.vector.memset(zero_sb, 0.0)

            wvf, wotf = [], []
            for k in range(2):
                wv_f = singles.tile([P, H], F32R, tag=f"wvf{k}")
                nc.scalar.dma_start(out=wv_f, in_=wv[k * P : (k + 1) * P, :])
                wvf.append(wv_f)
                wot_f = singles.tile([P, H], F32R, tag=f"wotf{k}")
                nc.scalar.dma_start(out=wot_f, in_=wot[k * P : (k + 1) * P, :])
                wotf.append(wot_f)

            def emit_a(xx, k):
                """Emit the 4 prefix matmuls of one block; returns psa tile."""
                psa = psuma_pool.tile([P, 2 * BT], F32, tag="psumA")
                for hf in range(2):
                    nc.tensor.matmul(
                        psa[:, hf * BT : hf * BT + BT],
                        lhsT=xx[:, k, 0, hf * P : (hf + 1) * P],
                        rhs=rmat_sb[:, 0:BT],
                        start=True, stop=False,
                    )
                    nc.tensor.matmul(
                        psa[:, hf * BT : hf * BT + BT],
                        lhsT=xx[:, k, 1, hf * P : (hf + 1) * P],
                        rhs=rmat_sb[:, BT : 2 * BT],
                        start=False, stop=True,
                    )
                return psa

            # emit superchunk 0's prefix matmuls BEFORE the W2 matmuls: PE is
            # in-order, and W2's weights (wv/wot) land after the first x data,
            # so this lets PE start ~5us earlier
            psa_pending = {}
            for k in range(KSC):
                psa_pending[(0, k)] = emit_a(xx_pending[0], k)

            # W2[h,h'] = sum_g Wv[g,h] * WoT[g,h'] ; h-halves in partitions
            w2sb = []
            for mh in range(2):
                pw = psump_pool.tile([P, 2 * H], F32, tag="psumP")
                nc.tensor.matmul(
                    pw[:, 0:H], lhsT=wvf[0][:, mh * P : (mh + 1) * P],
                    rhs=wotf[0][:, :], start=True, stop=False,
                )
                nc.tensor.matmul(
                    pw[:, 0:H], lhsT=wvf[1][:, mh * P : (mh + 1) * P],
                    rhs=wotf[1][:, :], start=False, stop=True,
                )
                w2 = singles.tile([P, H], F32R, tag=f"w2sb{mh}")
                nc.scalar.copy(w2, pw[:, 0:H])
                w2sb.append(w2)

            # ---- main loop ----
            pre_iter = 3  # superchunks already prefetched (batch 0)
            for b in range(BPC):
                prev_pref = [None, None]  # prefT of previous block per h-half
                for s in range(NSUPER):
                    if b == 0 and s < pre_iter:
                        xx = xx_pending[s]
                    else:
                        xx = xpool.tile([P, KSC, 2, H], F32R, tag="xx")
                        nc.sync.dma_start(out=xx, in_=xr[b, s])
                    ot = opool.tile([P, KSC, 2, H], F32, tag="ot")
                    for k in range(KSC):
                        g = s * KSC + k  # global block idx within batch

                        # block prefix, transposed: psa[:, hf*BT+i] per h-half
                        if b == 0 and (s, k) in psa_pending:
                            psa = psa_pending.pop((s, k))
                        else:
                            psa = emit_a(xx, k)

                        # carry add on ACT (both halves; chains stay ACT-local);
                        # the running total IS the previous block's column 255
                        pref = []
                        for hf in range(2):
                            pr = prefpool.tile([P, BT], F32R, tag=f"pref{hf}")
                            if g == 0:
                                old = zero_sb[:, 0:1]
                            else:
                                old = prev_pref[hf][:, BT - 1 : BT].bitcast(F32)
                            nc.scalar.activation(
                                pr, psa[:, hf * BT : hf * BT + BT],
                                AF.Identity, bias=old,
                            )
                            pref.append(pr)
                        prev_pref = pref

                        # projection back to token-major (even/odd token groups
                        # in one PSUM bank) + fused scale+residual on DVE
                        pp = psump_pool.tile([P, 2 * H], F32, tag="psumP")
                        for t in range(2):
                            nc.tensor.matmul(
                                pp[:, t * H : (t + 1) * H],
                                lhsT=pref[0][:, t : BT : 2],
                                rhs=w2sb[0][:, :],
                                start=True, stop=False,
                            )
                            nc.tensor.matmul(
                                pp[:, t * H : (t + 1) * H],
                                lhsT=pref[1][:, t : BT : 2],
                                rhs=w2sb[1][:, :],
                                start=False, stop=True,
                            )
                        for t in range(2):
                            nc.vector.scalar_tensor_tensor(
                                ot[:, k, t, :],
                                pp[:, t * H : (t + 1) * H],
                                recips_sb[:, 2 * g + t : 2 * g + t + 1],
                                xx[:, k, t, :].bitcast(F32),
                                ALU.mult,
                                ALU.add,
                            )
                        # out-DMA per block (256 KiB) on the idle GpSimd SWDGE
                        # ring: no HOL blocking of the input stream on the
                        # sync ring, and a short drain tail on the last block
                        nc.gpsimd.dma_start(
                            out=outr[b, s][:, k], in_=ot[:, k]
                        )
    nc.finalize()
    return nc


_NC_CACHE = None


def _get_nc() -> bass.Bass:
    global _NC_CACHE
    if _NC_CACHE is None:
        _NC_CACHE = build()
    return _NC_CACHE


def _make_in_maps(x, Wv, Wo):
    x = np.ascontiguousarray(np.asarray(x, dtype=np.float32))
    Wv = np.ascontiguousarray(np.asarray(Wv, dtype=np.float32))
    WoT = np.ascontiguousarray(np.asarray(Wo, dtype=np.float32).T)
    i_idx = np.arange(BT)[None, :]
    p_idx = np.arange(P)[:, None]
    r_even = (2 * p_idx <= i_idx).astype(np.float32)
    r_odd = (2 * p_idx + 1 <= i_idx).astype(np.float32)
    rmat_np = np.concatenate([r_even, r_odd], axis=1)  # [128, 512]
    # recips[p, 2g+t] = 1/(256g + 2p + t + 1)
    g_idx = np.arange(NBLOCK)[None, :]
    rec = np.zeros((P, 2 * NBLOCK), dtype=np.float32)
    rec[:, 0::2] = 1.0 / (256 * g_idx + 2 * p_idx + 1)
    rec[:, 1::2] = 1.0 / (256 * g_idx + 2 * p_idx + 2)
    return [
        {
            "x": x[BPC * i : BPC * (i + 1)],
            "wv": Wv,
            "wot": WoT,
            "rmat": rmat_np,
            "recips": rec,
        }
        for i in range(NCORES)
    ]


def run(x, Wv, Wo, trace: bool = False):
    """Run the SPMD kernel; returns (full_output, BassKernelResults)."""
    nc = _get_nc()
    in_maps = _make_in_maps(x, Wv, Wo)
    bres = run_bass_kernel_spmd(
        nc, in_maps, core_ids=list(range(NCORES)), trace=trace
    )
    full = np.concatenate(
        [np.asarray(bres.results[i]["out"]) for i in range(NCORES)], axis=0
    )
    return full, bres


def kernel(x, Wv, Wo) -> np.ndarray:
    return run(x, Wv, Wo)[0]


# revision 13
# speedup vs baseline: 1.0916x; 1.0743x over previous
"""Trainium2 Bass kernel for AnalyzableUniformAttention.

Reference computation (per batch element):
    v     = (x @ Wv.T) @ Wo.T
    mixed = T @ v            with T[i,j] = 1/(i+1) for j<=i  (causal cumulative mean)
    out   = x + mixed

Key algebraic restructure: T acts on the sequence dim, the projections act on
the feature dim, so they commute:
    out = x + D^-1 @ (tril @ x) @ W2      with W2 = Wv.T @ Wo.T, D = diag(i+1)

This halves the matmul FLOPs (one HxH projection instead of two) and lets the
prefix-sum (tril @ x) be computed block-wise with staircase matmuls plus an
f32 per-block running-total carry.

Sharding: pure data-parallel over batch B=16 -> 2 batch elements per core on
8 NeuronCores; the HxH weights are replicated. No collectives.

Per-core schedule (L=4096 tokens, H=256), all matmuls in float32r (full-rate
when the moving dim is 256):
  - tokens processed in blocks of 256; partition p holds the two ADJACENT
    tokens {256g+2p, 256g+2p+1} of block g, so every DMA descriptor moves a
    2 KiB contiguous HBM run. DMA at 1 MiB granularity (4 blocks), in-DMAs
    issued on the sync HWDGE ring, out-DMAs on the (otherwise idle) GpSimd
    SWDGE ring so they never head-of-line-block the input stream.
  - block prefix:  psa[:, hf*256+i] = sum_p x[2p+tt, h] * R_tt[p, i] for the
    even/odd token groups tt (R_tt[p,i] = [2p+tt <= i]), accumulated in one
    PSUM bank; psa column i is block token i in order.
  - carry (ACT):   prefT = psa + carry[h]; the carry scalar is simply the
                   previous block's prefT column 255 (running total).
  - projection:    psP_t[i, h'] = sum_h prefT[h, 2i+t] * W2[h, h'] via
                   stride-2 lhsT column slices, so psP_t partition p is block
                   token 2p+t — matching the DMA-friendly output layout.
  - scale+residual fused on DVE: out = (psP * recip_i) + x.
"""

import sys

import numpy as np

sys.path.insert(0, "/opt/trn_rl_repo")

import concourse.bacc as bacc  # noqa: E402
import concourse.bass as bass  # noqa: E402
import concourse.tile as tile  # noqa: E402
from concourse import mybir  # noqa: E402
from concourse.bass_utils import run_bass_kernel_spmd  # noqa: E402

B, L, H = 16, 4096, 256
NCORES = 8
BPC = B // NCORES  # batch elements per core
P = 128  # partitions
BT = 2 * P  # tokens per block (256)
NBLOCK = L // BT  # 16 blocks per batch element
KSC = 2  # blocks per superchunk (512 KiB DMA granularity)
NSUPER = NBLOCK // KSC

F32 = mybir.dt.float32
F32R = mybir.dt.float32r
BF16 = mybir.dt.bfloat16
AF = mybir.ActivationFunctionType
ALU = mybir.AluOpType


def build() -> bass.Bass:
    nc = bacc.Bacc()
    x = nc.declare_dram_parameter("x", [BPC, L, H], F32R, isOutput=False)
    wv = nc.declare_dram_parameter("wv", [H, H], F32R, isOutput=False)
    wot = nc.declare_dram_parameter("wot", [H, H], F32R, isOutput=False)
    # rmat = [R_even | R_odd]: R_tt[p, i] = 1 iff 2p+tt <= i
    rmat = nc.declare_dram_parameter("rmat", [P, 2 * BT], F32R, isOutput=False)
    # recips[p, 2g+t] = 1/(256g + 2p + t + 1)
    recips = nc.declare_dram_parameter("recips", [P, 2 * NBLOCK], F32, isOutput=False)
    out = nc.declare_dram_parameter("out", [BPC, L, H], F32, isOutput=True)

    with tile.TileContext(nc) as tc:
        with (
            tc.tile_pool(name="singles", bufs=1) as singles,
            tc.tile_pool(name="xpool", bufs=12) as xpool,
            tc.tile_pool(name="opool", bufs=6) as opool,
            tc.tile_pool(name="prefpool", bufs=6) as prefpool,
            tc.tile_pool(name="psumA", bufs=4, space="PSUM") as psuma_pool,
            tc.tile_pool(name="psumP", bufs=4, space="PSUM") as psump_pool,
        ):
            # token t = s*1024 + k*256 + 2p + two
            xr = x[:].rearrange("b (s k p two) h -> b s p k two h", k=KSC, p=P, two=2)
            outr = out[:].rearrange(
                "b (s k p two) h -> b s p k two h", k=KSC, p=P, two=2
            )

            # ---- prefetch the first two superchunks before anything else so
            # compute can start as soon as the constants land ----
            xx_pending = []
            for pre in range(3):
                xx = xpool.tile([P, KSC, 2, H], F32R, tag="xx")
                nc.sync.dma_start(out=xx, in_=xr[0, pre])
                xx_pending.append(xx)

            # ---- one-time: constants + fused weight W2 = Wv.T @ Wo.T ----
            rmat_sb = singles.tile([P, 2 * BT], F32R, tag="rmat")
            nc.scalar.dma_start(out=rmat_sb, in_=rmat[:, :])
            recips_sb = singles.tile([P, 2 * NBLOCK], F32, tag="recips")
            nc.scalar.dma_start(out=recips_sb, in_=recips[:, :])
            zero_sb = singles.tile([P, 1], F32, tag="zero")
            nc.vector.memset(zero_sb, 0.0)

            wvf, wotf = [], []
            for k in range(2):
                wv_f = singles.tile([P, H], F32R, tag=f"wvf{k}")
                nc.scalar.dma_start(out=wv_f, in_=wv[k * P : (k + 1) * P, :])
                wvf.append(wv_f)
                wot_f = singles.tile([P, H], F32R, tag=f"wotf{k}")
                nc.scalar.dma_start(out=wot_f, in_=wot[k * P : (k + 1) * P, :])
                wotf.append(wot_f)

            def emit_a(xx, k):
                """Emit the 4 prefix matmuls of one block; returns psa tile."""
                psa = psuma_pool.tile([P, 2 * BT], F32, tag="psumA")
                for hf in range(2):
                    nc.tensor.matmul(
                        psa[:, hf * BT : hf * BT + BT],
                        lhsT=xx[:, k, 0, hf * P : (hf + 1) * P],
                        rhs=rmat_sb[:, 0:BT],
                        start=True, stop=False,
                    )
                    nc.tensor.matmul(
                        psa[:, hf * BT : hf * BT + BT],
                        lhsT=xx[:, k, 1, hf * P : (hf + 1) * P],
                        rhs=rmat_sb[:, BT : 2 * BT],
                        start=False, stop=True,
                    )
                return psa

            # emit superchunk 0's prefix matmuls BEFORE the W2 matmuls: PE is
            # in-order, and W2's weights (wv/wot) land after the first x data,
            # so this lets PE start ~5us earlier
            psa_pending = {}
            for k in range(KSC):
                psa_pending[(0, k)] = emit_a(xx_pending[0], k)

            # W2[h,h'] = sum_g Wv[g,h] * WoT[g,h'] ; h-halves in partitions
            w2sb = []
            for mh in range(2):
                pw = psump_pool.tile([P, 2 * H], F32, tag="psumP")
                nc.tensor.matmul(
                    pw[:, 0:H], lhsT=wvf[0][:, mh * P : (mh + 1) * P],
                    rhs=wotf[0][:, :], start=True, stop=False,
                )
                nc.tensor.matmul(
                    pw[:, 0:H], lhsT=wvf[1][:, mh * P : (mh + 1) * P],
                    rhs=wotf[1][:, :], start=False, stop=True,
                )
                w2 = singles.tile([P, H], BF16, tag=f"w2sb{mh}")
                nc.scalar.copy(w2, pw[:, 0:H])
                w2sb.append(w2)

            # ---- main loop ----
            pre_iter = 3  # superchunks already prefetched (batch 0)
            for b in range(BPC):
                prev_pref = [None, None]  # prefT of previous block per h-half
                for s in range(NSUPER):
                    if b == 0 and s < pre_iter:
                        xx = xx_pending[s]
                    else:
                        xx = xpool.tile([P, KSC, 2, H], F32R, tag="xx")
                        nc.sync.dma_start(out=xx, in_=xr[b, s])
                    ot = opool.tile([P, KSC, 2, H], F32, tag="ot")
                    for k in range(KSC):
                        g = s * KSC + k  # global block idx within batch

                        # block prefix, transposed: psa[:, hf*BT+i] per h-half
                        if b == 0 and (s, k) in psa_pending:
                            psa = psa_pending.pop((s, k))
                        else:
                            psa = emit_a(xx, k)

                        # carry add on ACT (both halves; chains stay ACT-local);
                        # the running total IS the previous block's column 255
                        pref = []
                        for hf in range(2):
                            pr = prefpool.tile([P, BT], BF16, tag=f"pref{hf}")
                            if g == 0:
                                old = zero_sb[:, 0:1]
                            else:
                                old = prev_pref[hf][:, BT - 1 : BT]
                            nc.scalar.activation(
                                pr, psa[:, hf * BT : hf * BT + BT],
                                AF.Identity, bias=old,
                            )
                            pref.append(pr)
                        prev_pref = pref

                        # projection back to token-major (even/odd token groups
                        # in one PSUM bank) + fused scale+residual on DVE
                        pp = psump_pool.tile([P, 2 * H], F32, tag="psumP")
                        for t in range(2):
                            nc.tensor.matmul(
                                pp[:, t * H : (t + 1) * H],
                                lhsT=pref[0][:, t : BT : 2],
                                rhs=w2sb[0][:, :],
                                start=True, stop=False,
                            )
                            nc.tensor.matmul(
                                pp[:, t * H : (t + 1) * H],
                                lhsT=pref[1][:, t : BT : 2],
                                rhs=w2sb[1][:, :],
                                start=False, stop=True,
                            )
                        for t in range(2):
                            nc.vector.scalar_tensor_tensor(
                                ot[:, k, t, :],
                                pp[:, t * H : (t + 1) * H],
                                recips_sb[:, 2 * g + t : 2 * g + t + 1],
                                xx[:, k, t, :].bitcast(F32),
                                ALU.mult,
                                ALU.add,
                            )
                        # out-DMA per block (256 KiB) on the idle GpSimd SWDGE
                        # ring: no HOL blocking of the input stream on the
                        # sync ring, and a short drain tail on the last block
                        nc.gpsimd.dma_start(
                            out=outr[b, s][:, k], in_=ot[:, k]
                        )
    nc.finalize()
    return nc


_NC_CACHE = None


def _get_nc() -> bass.Bass:
    global _NC_CACHE
    if _NC_CACHE is None:
        _NC_CACHE = build()
    return _NC_CACHE


def _make_in_maps(x, Wv, Wo):
    x = np.ascontiguousarray(np.asarray(x, dtype=np.float32))
    Wv = np.ascontiguousarray(np.asarray(Wv, dtype=np.float32))
    WoT = np.ascontiguousarray(np.asarray(Wo, dtype=np.float32).T)
    i_idx = np.arange(BT)[None, :]
    p_idx = np.arange(P)[:, None]
    r_even = (2 * p_idx <= i_idx).astype(np.float32)
    r_odd = (2 * p_idx + 1 <= i_idx).astype(np.float32)
    rmat_np = np.concatenate([r_even, r_odd], axis=1)  # [128, 512]
    # recips[p, 2g+t] = 1/(256g + 2p + t + 1)
    g_idx = np.arange(NBLOCK)[None, :]
    rec = np.zeros((P, 2 * NBLOCK), dtype=np.float32)
    rec[:, 0::2] = 1.0 / (256 * g_idx + 2 * p_idx + 1)
    rec[:, 1::2] = 1.0 / (256 * g_idx + 2 * p_idx + 2)
    return [
        {
            "x": x[BPC * i : BPC * (i + 1)],
            "wv": Wv,
            "wot": WoT,
            "rmat": rmat_np,
            "recips": rec,
        }
        for i in range(NCORES)
    ]


def run(x, Wv, Wo, trace: bool = False):
    """Run the SPMD kernel; returns (full_output, BassKernelResults)."""
    nc = _get_nc()
    in_maps = _make_in_maps(x, Wv, Wo)
    bres = run_bass_kernel_spmd(
        nc, in_maps, core_ids=list(range(NCORES)), trace=trace
    )
    full = np.concatenate(
        [np.asarray(bres.results[i]["out"]) for i in range(NCORES)], axis=0
    )
    return full, bres


def kernel(x, Wv, Wo) -> np.ndarray:
    return run(x, Wv, Wo)[0]
